# revision 17
# baseline (speedup 1.0000x reference)
"""Trainium2 Bass kernel for nn_CNNModel_82222853915196.

Model (per utterance x: (64, 512)):
  multiscale patch features (h in {8,16,32,64}) -> feats (8192,)
  out[t, :] = Wfc @ concat([x[:, t], feats]) + bfc

Factorization: feats is broadcast over t, so
  out = x.T @ Wfc1.T + 1*(Wfc2 @ feats).T + 1*cconst.T
with all feature-bias terms folded into cconst on the host.

Feature contraction: masked stationary weights over the full 64-row
contraction fuse all row offsets k into the matmul M dim; two within-row
offsets j are fused into K=128 via a second, one-column-shifted copy of x
in SBUF partitions 64..127.

Each scale's feature matmuls are arranged so the staged fp16 tile is
[ (u, kt') rows | fp cols ] -- a pure 2D transpose of the [fp | (u, kt)]
layout the C matmul needs -- so a single SBUF->SBUF xbar-transpose DMA per
scale replaces any DRAM scatter/gather round trip.

C rides into the frames matmul as a second accumulating matmul (K=4,
one-hot utterance selector x C rows), so no cross-partition copies of C
are needed; cconst is a host-prefilled ones-row (K=65 main matmul).

DMA count is kept minimal (the rings serialize at ~2us per dma_start):
the big streams are split over both HWDGE rings, small weights ride in one
merged upload, outputs go out per-utterance on the SWDGE ring.

Everything runs in fp16 except PSUM accumulation (fp32); output is written
fp16 and cast to fp32 on the host. Overall rel err ~5e-4 vs tolerance 2e-2.

Sharding: pure data parallel - 32 utterances -> 8 cores x 4. Weights
replicated; no cross-core communication (collectives have ~10-100us fixed
cost in this environment and are a net loss).
"""

import os
import sys
from contextlib import ExitStack

import numpy as np

for _p in ("/opt/trn_rl_repo", "/root/.axon_site/_ro/trn_rl_repo"):
    if os.path.isdir(_p) and _p not in sys.path:
        sys.path.insert(0, _p)

import concourse.bass as bass
import concourse.tile as tile
from concourse import bacc, mybir
from concourse.bass_utils import run_bass_kernel_spmd

NCORES = 8
NUTT = 4                 # utterances per core
T = 512
F = 64
OUT = 400
W = NUTT * T             # 2048, free width of the x tile
FP32 = mybir.dt.float32
FP16 = mybir.dt.float16
NPF16 = np.float16


# ---------------------------------------------------------------------------
# host-side weight preparation
# ---------------------------------------------------------------------------

def _build_devindex():
    """dev[kt, fp] = reference flat feature index m in [0, 8192).

    Device feats layout (scale regions of 16 kt each):
      h=8 : kt = ph,        fp = pl*32 + k*4 + o    (p = ph*4 + pl)
      h=16: kt = 16 + ph,   fp = pl*64 + k*16 + o   (p = ph*2 + pl)
      h=32: kt = 32 + p,    fp = k*64 + o
      h=64: kt = 48+p*2+oh, fp = ol                 (o = oh*128 + ol)
    """
    dev = np.full((64, 128), -1, dtype=np.int64)
    for ph in range(16):
        for pl in range(4):
            for k in range(8):
                for o in range(4):
                    dev[ph, pl * 32 + k * 4 + o] = (k * 64 + ph * 4 + pl) * 4 + o
    for ph in range(16):
        for pl in range(2):
            for k in range(4):
                for o in range(16):
                    dev[16 + ph, pl * 64 + k * 16 + o] = \
                        2048 + (k * 32 + ph * 2 + pl) * 16 + o
    for p in range(16):
        for k in range(2):
            for o in range(64):
                dev[32 + p, k * 64 + o] = 4096 + (k * 16 + p) * 64 + o
    for p in range(8):
        for o in range(256):
            dev[48 + p * 2 + o // 128, o % 128] = 6144 + p * 256 + o
    assert dev.min() >= 0
    return dev


def _masked2(Wh, nk, h, no):
    """w2[(jo, r), j0*(nk*no) + k*no + o] = Wh[k, o, (r-k)*h + 2*j0 + jo]
    for 0 <= r-k < h, else 0."""
    f32 = np.float32
    w = np.zeros((2, 64, h // 2, nk * no), dtype=f32)
    for k in range(nk):
        Wk = np.asarray(Wh[k], f32).reshape(no, h, h)      # [o, i, j]
        for jo in range(2):
            # [i, j0, o]
            w[jo, k:k + h, :, k * no:(k + 1) * no] = \
                Wk[:, :, jo::2].transpose(1, 2, 0)
    return w.reshape(128, (h // 2) * nk * no)


def host_prep(W8, b8, W16, b16, W32, b32, W64, b64, Wfc, bfc):
    f32 = np.float32
    W64 = np.asarray(W64, f32)
    Wfc = np.asarray(Wfc, f32)
    b8 = np.asarray(b8, f32); b16 = np.asarray(b16, f32)
    b32 = np.asarray(b32, f32); b64 = np.asarray(b64, f32)
    bfc = np.asarray(bfc, f32)

    w8j2 = _masked2(W8, 8, 8, 4)        # [128, 128]
    w16j2 = _masked2(W16, 4, 16, 16)    # [128, 512]
    w32j2 = _masked2(W32, 2, 32, 64)    # [128, 2048]
    wsmall = np.concatenate([w8j2, w16j2, w32j2], axis=1)  # [128, 2688]
    # w64w2[(jo,i), j0*256+o] = W64[o, i*64 + 2*j0 + jo]
    w64w2 = np.ascontiguousarray(
        W64.reshape(256, 64, 32, 2).transpose(3, 1, 2, 0).reshape(128, 8192))

    dev = _build_devindex()
    Wfc2 = Wfc[:, 64:]
    # wfc2c[fp, kt*OUT + o] = Wfc2[o, dev[kt, fp]]
    wfc2c = np.ascontiguousarray(
        Wfc2[:, dev.reshape(-1)].T.reshape(64, 128, OUT)
        .transpose(1, 0, 2).reshape(128, 64 * OUT))

    fb = np.zeros(8192, dtype=np.float64)
    fb[0:2048] = np.broadcast_to(b8[:, None, :], (8, 64, 4)).reshape(-1)
    fb[2048:4096] = np.broadcast_to(b16[:, None, :], (4, 32, 16)).reshape(-1)
    fb[4096:6144] = np.broadcast_to(b32[:, None, :], (2, 16, 64)).reshape(-1)
    fb[6144:8192] = np.broadcast_to(b64[None, :], (8, 256)).reshape(-1)
    cconst = (Wfc2.astype(np.float64) @ fb + bfc.astype(np.float64)).astype(f32)

    # frames moving operand: rows 0..63 = wfc1 (tiled per utt), row 64 = cconst
    wfc1c = np.zeros((65, NUTT * OUT), dtype=f32)
    wfc1c[0:64] = np.tile(Wfc[:, :64].T, (1, NUTT))
    wfc1c[64] = np.tile(cconst, NUTT)

    e4 = np.zeros((NUTT, NUTT * 128), dtype=f32)
    for u in range(NUTT):
        e4[u, u * 128:(u + 1) * 128] = 1.0

    return {
        "e4": e4.astype(NPF16),
        "wsmall": np.ascontiguousarray(wsmall.astype(NPF16)),
        "w64w2": w64w2.astype(NPF16),
        "wfc2c": wfc2c.astype(NPF16),
        "wfc1c": np.ascontiguousarray(wfc1c.astype(NPF16)),
    }


# ---------------------------------------------------------------------------
# device program
# ---------------------------------------------------------------------------

def build_program(trace_sim=False):
    nc = bacc.Bacc("TRN2", target_bir_lowering=False, debug=False)

    dram = dict(
        xdup=nc.dram_tensor("xdup", [128, W], FP16, kind="ExternalInput"),
        wsmall=nc.dram_tensor("wsmall", [128, 2688], FP16, kind="ExternalInput"),
        w64w2=nc.dram_tensor("w64w2", [128, 8192], FP16, kind="ExternalInput"),
        wfc2c=nc.dram_tensor("wfc2c", [128, 64 * OUT], FP16, kind="ExternalInput"),
        wfc1c=nc.dram_tensor("wfc1c", [65, NUTT * OUT], FP16, kind="ExternalInput"),
        e4=nc.dram_tensor("e4", [NUTT, NUTT * 128], FP16, kind="ExternalInput"),
        out=nc.dram_tensor("out", [NUTT, 128, 4 * OUT], FP16, kind="ExternalOutput"),
    )

    with tile.TileContext(nc, trace_sim=trace_sim) as tc:
        with ExitStack() as ctx:
            _emit(nc, tc, ctx, dram)

    nc.compile()
    return nc


def _emit(nc, tc, ctx, dram):
    scalar_dma = nc.scalar.dma_start
    gpsimd_dma = nc.gpsimd.dma_start
    sync_dma = nc.sync.dma_start

    const = ctx.enter_context(tc.tile_pool(name="const", bufs=1))
    stg = ctx.enter_context(tc.tile_pool(name="stg", bufs=2))
    wfc2p = ctx.enter_context(tc.tile_pool(name="wfc2p", bufs=4))
    outp = ctx.enter_context(tc.tile_pool(name="outp", bufs=2))
    ps = ctx.enter_context(tc.tile_pool(name="ps", bufs=2, space="PSUM"))
    psc = ctx.enter_context(tc.tile_pool(name="psc", bufs=1, space="PSUM"))
    psf = ctx.enter_context(tc.tile_pool(name="psf", bufs=3, space="PSUM"))

    CH = 16  # wfc2 kt per streamed chunk (one chunk per scale region)

    # ---- ring layout (each dma_start costs ~2us of serialized ring time):
    #  sync   : xdup-lo, w64-h1, xbar transposes, chunks 0, 1
    #  scalar : xdup-hi, w64-h2, chunks 2, 3
    #  gpsimd : wsmall, wfc1c, out x4
    xdup = const.tile([128, W], FP16, tag="xdup")
    sync_dma(xdup[0:64, :],
             bass.AP(tensor=dram["xdup"], offset=0, ap=[[W, 64], [1, W]]))
    scalar_dma(xdup[64:128, :],
               bass.AP(tensor=dram["xdup"], offset=64 * W, ap=[[W, 64], [1, W]]))

    w64w2 = const.tile([128, 8192], FP16, tag="w64w2")
    sync_dma(w64w2[:, 0:4096],
             bass.AP(tensor=dram["w64w2"], offset=0, ap=[[8192, 128], [1, 4096]]))
    scalar_dma(w64w2[:, 4096:8192],
               bass.AP(tensor=dram["w64w2"], offset=4096,
                       ap=[[8192, 128], [1, 4096]]))

    wsmall = const.tile([128, 2688], FP16, tag="wsmall")
    gpsimd_dma(wsmall[:], dram["wsmall"].ap())
    w8j2 = wsmall[:, 0:128]
    w16j2 = wsmall[:, 128:640]
    w32j2 = wsmall[:, 640:2688]

    # frames moving operand: rows 0..63 wfc1, row 64 cconst (host-prefilled)
    rhs65 = const.tile([65, NUTT * OUT], FP16, tag="rhs65")
    gpsimd_dma(rhs65[:], dram["wfc1c"].ap())

    def load_wfc2_chunk(ch, dma):
        chunk = wfc2p.tile([128, CH * OUT], FP16, tag="wfc2chunk", bufs=4)
        dma(chunk[:],
            bass.AP(tensor=dram["wfc2c"], offset=ch * CH * OUT,
                    ap=[[64 * OUT, 128], [1, CH * OUT]]))
        return chunk

    chunks = [None] * 4
    chunks[2] = load_wfc2_chunk(2, scalar_dma)
    chunks[3] = load_wfc2_chunk(3, scalar_dma)

    # one-hot utterance selector for the C-broadcast matmul (K=4)
    e4 = const.tile([NUTT, NUTT * 128], FP16, tag="e4")
    gpsimd_dma(e4[:], dram["e4"].ap())

    feats = const.tile([128, 256], FP16, tag="feats")
    fv = feats.rearrange("f (s u k) -> f s u k", s=4, u=4)
    cps = psc.tile([NUTT, OUT], FP32, tag="cps")

    cmm_n = [0]

    def cmms(b, fv_slice):
        """C matmuls for region b: 16 kts, M=4 utts, N=400, PSUM-accum."""
        chunk = chunks[b]
        for k in range(16):
            nc.tensor.matmul(cps[:], fv_slice(k),
                             chunk[:, k * OUT:(k + 1) * OUT],
                             start=(cmm_n[0] == 0), stop=(cmm_n[0] == 63))
            cmm_n[0] += 1

    def xbar(b, st):
        """SBUF->SBUF xbar transpose: st [64 (u,kt'), 128 fp] ->
        feats[:, b*64:(b+1)*64]."""
        nc.sync.dma_start_transpose(feats[:, b * 64:(b + 1) * 64], st[:])

    xv = xdup[:].rearrange("i (u t) -> i u t", u=NUTT)

    # ---- scale h=8: rows (u, ph), cols (pl, q). 32 MMs K=128 M=32 N=32.
    acc = ps.tile([64, 128], FP32, tag="featps")
    x8 = xv.rearrange("i u (ph pl j) -> i u ph pl j", pl=4, j=8)
    for half in range(2):
        for pl in range(4):
            for j0 in range(4):
                nc.tensor.matmul(
                    acc[half * 32:(half + 1) * 32, pl * 32:(pl + 1) * 32],
                    x8[:, 2 * half:2 * half + 2, :, pl, 2 * j0],
                    w8j2[:, j0 * 32:(j0 + 1) * 32],
                    start=(j0 == 0), stop=(j0 == 3))
    st8 = stg.tile([64, 128], FP16, tag="featst")
    nc.vector.tensor_copy(st8[:], acc[:])
    xbar(0, st8)

    # ---- scale h=16: rows (u, ph), cols (pl, q). 16 MMs K=128 M=64 N=64.
    acc = ps.tile([64, 128], FP32, tag="featps")
    x16 = xv.rearrange("i u (ph pl j) -> i u ph pl j", pl=2, j=16)
    for pl in range(2):
        for j0 in range(8):
            nc.tensor.matmul(
                acc[:, pl * 64:(pl + 1) * 64],
                x16[:, :, :, pl, 2 * j0],
                w16j2[:, j0 * 64:(j0 + 1) * 64],
                start=(j0 == 0), stop=(j0 == 7))
    st16 = stg.tile([64, 128], FP16, tag="featst")
    nc.vector.tensor_copy(st16[:], acc[:])
    xbar(1, st16)

    chunks[0] = load_wfc2_chunk(0, sync_dma)

    # ---- scale h=32: rows (u, p), cols q. 16 MMs K=128 M=64 N=128.
    acc = ps.tile([64, 128], FP32, tag="featps")
    x32 = xv.rearrange("i u (p j) -> i u p j", j=32)
    for j0 in range(16):
        nc.tensor.matmul(acc[:], x32[:, :, :, 2 * j0],
                         w32j2[:, j0 * 128:(j0 + 1) * 128],
                         start=(j0 == 0), stop=(j0 == 15))
    st32 = stg.tile([64, 128], FP16, tag="featst")
    nc.vector.tensor_copy(st32[:], acc[:])
    xbar(2, st32)

    # frames stationary: rows 0..63 = x, row 64 = ones (DVE idle window)
    x65 = const.tile([65, W], FP16, tag="x65")
    nc.vector.tensor_copy(x65[0:64, :], xdup[0:64, :])
    nc.vector.memset(x65[64:65, :], 1.0)

    # ---- scale h=64: rows (oh, u, p), cols ol. 64 MMs K=128 M=32 N=128.
    acc = ps.tile([64, 128], FP32, tag="featps")
    x64 = xv.rearrange("i u (p j) -> i u p j", j=64)
    for oh in range(2):
        for j0 in range(32):
            nc.tensor.matmul(
                acc[oh * 32:(oh + 1) * 32, :],
                x64[:, :, :, 2 * j0],
                w64w2[:, j0 * 256 + oh * 128: j0 * 256 + (oh + 1) * 128],
                start=(j0 == 0), stop=(j0 == 31))
    st64 = stg.tile([64, 128], FP16, tag="featst")
    nc.vector.tensor_copy(st64[:], acc[:])
    xbar(3, st64)

    chunks[1] = load_wfc2_chunk(1, sync_dma)

    # region 3 cols are (oh, u, p): col(u, kt') = (kt'%2)*32 + u*8 + kt'//2
    fv3 = feats[:, 192:256].rearrange("f (oh u p) -> f oh u p", oh=2, u=4)
    # C matmuls in chunk-arrival order: 2 (scalar, early), 0, 3, 1
    cmms(2, lambda k: fv[:, 2, :, k])
    cmms(0, lambda k: fv[:, 0, :, k])
    cmms(3, lambda k: fv3[:, k % 2, :, k // 2])
    cmms(1, lambda k: fv[:, 1, :, k])

    # ---- frames matmul: out = x^T @ Wfc1^T + 1*cconst + onehot_u^T @ C
    csb = stg.tile([NUTT, OUT], FP16, tag="csb")
    nc.vector.tensor_copy(csb[:], cps[:])
    for u in range(NUTT):
        fsb = outp.tile([128, 4 * OUT], FP16, tag="framesout")
        for tc_i in range(4):
            fps = psf.tile([128, OUT], FP32, tag="framesps")
            nc.tensor.matmul(
                fps[:],
                x65[:, u * T + tc_i * 128: u * T + (tc_i + 1) * 128],
                rhs65[:, u * OUT:(u + 1) * OUT], start=True, stop=False)
            nc.tensor.matmul(
                fps[:], e4[:, u * 128:(u + 1) * 128], csb[:],
                start=False, stop=True)
            if tc_i % 2 == 0:
                nc.vector.tensor_copy(fsb[:, tc_i * OUT:(tc_i + 1) * OUT], fps[:])
            else:
                nc.scalar.activation(fsb[:, tc_i * OUT:(tc_i + 1) * OUT], fps[:],
                                     mybir.ActivationFunctionType.Copy)
        gpsimd_dma(
            bass.AP(tensor=dram["out"], offset=u * 128 * 4 * OUT,
                    ap=[[4 * OUT, 128], [1, 4 * OUT]]),
            fsb[:])


_NC_CACHE = None


def _get_nc():
    global _NC_CACHE
    if _NC_CACHE is None:
        _NC_CACHE = build_program()
    return _NC_CACHE


# ---------------------------------------------------------------------------
# entry point
# ---------------------------------------------------------------------------

def run(inputs, trace=False, **kw):
    nc = _get_nc()
    prep = host_prep(inputs["W8"], inputs["b8"], inputs["W16"], inputs["b16"],
                     inputs["W32"], inputs["b32"], inputs["W64"], inputs["b64"],
                     inputs["Wfc"], inputs["bfc"])
    batch = np.asarray(inputs["batch"], np.float32)
    in_maps = []
    for c in range(NCORES):
        x4 = batch[NUTT * c:NUTT * (c + 1)].transpose(1, 0, 2).reshape(F, W)
        xdup = np.zeros((128, W), dtype=NPF16)
        xdup[0:64] = x4.astype(NPF16)
        xdup[64:128, :W - 1] = x4[:, 1:].astype(NPF16)
        m = dict(prep)
        m["xdup"] = xdup
        in_maps.append(m)
    res = run_bass_kernel_spmd(nc, in_maps, core_ids=list(range(NCORES)),
                               trace=trace, **kw)
    # out[u, t, tc, o] -> rows u*512 + tc*128 + t
    out = np.concatenate(
        [r["out"].reshape(NUTT, 128, 4, OUT).transpose(0, 2, 1, 3)
         .reshape(NUTT * T, OUT) for r in res.results], axis=0)
    return out.astype(np.float32), res


def kernel(**inputs):
    out, _ = run(inputs)
    return out


# revision 19
# speedup vs baseline: 1.0966x; 1.0966x over previous
"""Trainium2 Bass kernel for nn_CNNModel_82222853915196.

Model (per utterance x: (64, 512)):
  multiscale patch features (h in {8,16,32,64}) -> feats (8192,)
  out[t, :] = Wfc @ concat([x[:, t], feats]) + bfc

Factorization: feats is broadcast over t, so
  out = x.T @ Wfc1.T + 1*(Wfc2 @ feats).T + 1*cconst.T
with all feature-bias terms folded into cconst on the host.

Feature contraction: masked stationary weights over the full 64-row
contraction fuse all row offsets k into the matmul M dim; two within-row
offsets j are fused into K=128 via a second, one-column-shifted copy of x
in SBUF partitions 64..127.

Each scale's feature matmuls are arranged so the staged fp16 tile is
[ (u, kt') rows | fp cols ] -- a pure 2D transpose of the [fp | (u, kt)]
layout the C matmul needs -- so a single SBUF->SBUF xbar-transpose DMA per
scale replaces any DRAM scatter/gather round trip.

C rides into the frames matmul as a second accumulating matmul (K=4,
one-hot utterance selector x C rows), so no cross-partition copies of C
are needed; cconst is a host-prefilled ones-row (K=65 main matmul).

DMA count is kept minimal (the rings serialize at ~2us per dma_start):
the big streams are split over both HWDGE rings, small weights ride in one
merged upload, outputs go out per-utterance on the SWDGE ring.

Everything runs in fp16 except PSUM accumulation (fp32); output is written
fp16 and cast to fp32 on the host. Overall rel err ~5e-4 vs tolerance 2e-2.

Sharding: pure data parallel - 32 utterances -> 8 cores x 4. Weights
replicated; no cross-core communication (collectives have ~10-100us fixed
cost in this environment and are a net loss).
"""

import os
import sys
from contextlib import ExitStack

import numpy as np

for _p in ("/opt/trn_rl_repo", "/root/.axon_site/_ro/trn_rl_repo"):
    if os.path.isdir(_p) and _p not in sys.path:
        sys.path.insert(0, _p)

import concourse.bass as bass
import concourse.tile as tile
from concourse import bacc, mybir
from concourse.bass_utils import run_bass_kernel_spmd

NCORES = 8
NUTT = 4                 # utterances per core
T = 512
F = 64
OUT = 400
W = NUTT * T             # 2048, free width of the x tile
FP32 = mybir.dt.float32
FP16 = mybir.dt.float16
NPF16 = np.float16


# ---------------------------------------------------------------------------
# host-side weight preparation
# ---------------------------------------------------------------------------

def _build_devindex():
    """dev[kt, fp] = reference flat feature index m in [0, 8192).

    Device feats layout (scale regions of 16 kt each):
      h=8 : kt = ph,        fp = pl*32 + k*4 + o    (p = ph*4 + pl)
      h=16: kt = 16 + ph,   fp = pl*64 + k*16 + o   (p = ph*2 + pl)
      h=32: kt = 32 + p,    fp = k*64 + o
      h=64: kt = 48+p*2+oh, fp = ol                 (o = oh*128 + ol)
    """
    dev = np.full((64, 128), -1, dtype=np.int64)
    for ph in range(16):
        for pl in range(4):
            for k in range(8):
                for o in range(4):
                    dev[ph, pl * 32 + k * 4 + o] = (k * 64 + ph * 4 + pl) * 4 + o
    for ph in range(16):
        for pl in range(2):
            for k in range(4):
                for o in range(16):
                    dev[16 + ph, pl * 64 + k * 16 + o] = \
                        2048 + (k * 32 + ph * 2 + pl) * 16 + o
    for p in range(16):
        for k in range(2):
            for o in range(64):
                dev[32 + p, k * 64 + o] = 4096 + (k * 16 + p) * 64 + o
    for p in range(8):
        for o in range(256):
            dev[48 + p * 2 + o // 128, o % 128] = 6144 + p * 256 + o
    assert dev.min() >= 0
    return dev


def _masked2(Wh, nk, h, no):
    """w2[(jo, r), j0*(nk*no) + k*no + o] = Wh[k, o, (r-k)*h + 2*j0 + jo]
    for 0 <= r-k < h, else 0."""
    f32 = np.float32
    w = np.zeros((2, 64, h // 2, nk * no), dtype=f32)
    for k in range(nk):
        Wk = np.asarray(Wh[k], f32).reshape(no, h, h)      # [o, i, j]
        for jo in range(2):
            # [i, j0, o]
            w[jo, k:k + h, :, k * no:(k + 1) * no] = \
                Wk[:, :, jo::2].transpose(1, 2, 0)
    return w.reshape(128, (h // 2) * nk * no)


def host_prep(W8, b8, W16, b16, W32, b32, W64, b64, Wfc, bfc):
    f32 = np.float32
    W64 = np.asarray(W64, f32)
    Wfc = np.asarray(Wfc, f32)
    b8 = np.asarray(b8, f32); b16 = np.asarray(b16, f32)
    b32 = np.asarray(b32, f32); b64 = np.asarray(b64, f32)
    bfc = np.asarray(bfc, f32)

    w8j2 = _masked2(W8, 8, 8, 4)        # [128, 128]
    w16j2 = _masked2(W16, 4, 16, 16)    # [128, 512]
    w32j2 = _masked2(W32, 2, 32, 64)    # [128, 2048]
    wsmall = np.concatenate([w8j2, w16j2, w32j2], axis=1)  # [128, 2688]
    # w64w2[(jo,i), j0*256+o] = W64[o, i*64 + 2*j0 + jo]
    w64w2 = np.ascontiguousarray(
        W64.reshape(256, 64, 32, 2).transpose(3, 1, 2, 0).reshape(128, 8192))

    dev = _build_devindex()
    Wfc2 = Wfc[:, 64:]
    # wfc2c[fp, kt*OUT + o] = Wfc2[o, dev[kt, fp]]
    wfc2c = np.ascontiguousarray(
        Wfc2[:, dev.reshape(-1)].T.reshape(64, 128, OUT)
        .transpose(1, 0, 2).reshape(128, 64 * OUT))

    fb = np.zeros(8192, dtype=np.float64)
    fb[0:2048] = np.broadcast_to(b8[:, None, :], (8, 64, 4)).reshape(-1)
    fb[2048:4096] = np.broadcast_to(b16[:, None, :], (4, 32, 16)).reshape(-1)
    fb[4096:6144] = np.broadcast_to(b32[:, None, :], (2, 16, 64)).reshape(-1)
    fb[6144:8192] = np.broadcast_to(b64[None, :], (8, 256)).reshape(-1)
    cconst = (Wfc2.astype(np.float64) @ fb + bfc.astype(np.float64)).astype(f32)

    # frames moving operand: rows 0..63 = wfc1 (tiled per utt), row 64 = cconst
    wfc1c = np.zeros((65, NUTT * OUT), dtype=f32)
    wfc1c[0:64] = np.tile(Wfc[:, :64].T, (1, NUTT))
    wfc1c[64] = np.tile(cconst, NUTT)

    e4 = np.zeros((NUTT, NUTT * 128), dtype=f32)
    for u in range(NUTT):
        e4[u, u * 128:(u + 1) * 128] = 1.0

    return {
        "e4": e4.astype(NPF16),
        "wsmall": np.ascontiguousarray(wsmall.astype(NPF16)),
        "w64w2": w64w2.astype(NPF16),
        "wfc2c": wfc2c.astype(NPF16),
        "wfc1c": np.ascontiguousarray(wfc1c.astype(NPF16)),
    }


XW_W = W + 2688          # xdup ++ wsmall, merged early upload


# ---------------------------------------------------------------------------
# device program
# ---------------------------------------------------------------------------

def build_program(trace_sim=False):
    nc = bacc.Bacc("TRN2", target_bir_lowering=False, debug=False)

    dram = dict(
        xw=nc.dram_tensor("xw", [128, XW_W], FP16, kind="ExternalInput"),
        w64w2=nc.dram_tensor("w64w2", [128, 8192], FP16, kind="ExternalInput"),
        wfc2c=nc.dram_tensor("wfc2c", [128, 64 * OUT], FP16, kind="ExternalInput"),
        wfc1c=nc.dram_tensor("wfc1c", [65, NUTT * OUT], FP16, kind="ExternalInput"),
        e4=nc.dram_tensor("e4", [NUTT, NUTT * 128], FP16, kind="ExternalInput"),
        out=nc.dram_tensor("out", [128, NUTT * 4 * OUT], FP16, kind="ExternalOutput"),
    )

    with tile.TileContext(nc, trace_sim=trace_sim) as tc:
        with ExitStack() as ctx:
            _emit(nc, tc, ctx, dram)

    nc.compile()
    return nc


def _emit(nc, tc, ctx, dram):
    scalar_dma = nc.scalar.dma_start
    gpsimd_dma = nc.gpsimd.dma_start
    sync_dma = nc.sync.dma_start

    const = ctx.enter_context(tc.tile_pool(name="const", bufs=1))
    stg = ctx.enter_context(tc.tile_pool(name="stg", bufs=2))
    wfc2p = ctx.enter_context(tc.tile_pool(name="wfc2p", bufs=4))
    outp = ctx.enter_context(tc.tile_pool(name="outp", bufs=2))
    ps = ctx.enter_context(tc.tile_pool(name="ps", bufs=2, space="PSUM"))
    psc = ctx.enter_context(tc.tile_pool(name="psc", bufs=1, space="PSUM"))
    psf = ctx.enter_context(tc.tile_pool(name="psf", bufs=3, space="PSUM"))

    CH = 16  # wfc2 kt per streamed chunk (one chunk per scale region)

    # ---- ring layout (HWDGE rings ~130-165 GB/s each; SWDGE ~340 GB/s on
    # big DMAs but ~2us serial setup each):
    #  gpsimd : xw (x + small weights, first), chunk0, chunk1, out (one DMA)
    #  sync   : w64-h1, xbar transposes, chunk2
    #  scalar : w64-h2, wfc1c, e4, chunk3
    xw = const.tile([128, XW_W], FP16, tag="xw")
    gpsimd_dma(xw[:], dram["xw"].ap())
    xdup = xw[:, 0:W]
    w8j2 = xw[:, W:W + 128]
    w16j2 = xw[:, W + 128:W + 640]
    w32j2 = xw[:, W + 640:W + 2688]

    w64w2 = const.tile([128, 8192], FP16, tag="w64w2")
    sync_dma(w64w2[:, 0:4096],
             bass.AP(tensor=dram["w64w2"], offset=0, ap=[[8192, 128], [1, 4096]]))
    scalar_dma(w64w2[:, 4096:8192],
               bass.AP(tensor=dram["w64w2"], offset=4096,
                       ap=[[8192, 128], [1, 4096]]))

    # frames moving operand: rows 0..63 wfc1, row 64 cconst (host-prefilled)
    rhs65 = const.tile([65, NUTT * OUT], FP16, tag="rhs65")
    scalar_dma(rhs65[:], dram["wfc1c"].ap())

    def load_wfc2_chunk(ch, dma):
        chunk = wfc2p.tile([128, CH * OUT], FP16, tag="wfc2chunk", bufs=4)
        dma(chunk[:],
            bass.AP(tensor=dram["wfc2c"], offset=ch * CH * OUT,
                    ap=[[64 * OUT, 128], [1, CH * OUT]]))
        return chunk

    chunks = [None] * 4
    chunks[0] = load_wfc2_chunk(0, gpsimd_dma)
    chunks[1] = load_wfc2_chunk(1, gpsimd_dma)

    # one-hot utterance selector for the C-broadcast matmul (K=4)
    e4 = const.tile([NUTT, NUTT * 128], FP16, tag="e4")
    scalar_dma(e4[:], dram["e4"].ap())

    feats = const.tile([128, 256], FP16, tag="feats")
    fv = feats.rearrange("f (s u k) -> f s u k", s=4, u=4)
    cps = psc.tile([NUTT, OUT], FP32, tag="cps")

    cmm_n = [0]

    def cmms(b, fv_slice):
        """C matmuls for region b: 16 kts, M=4 utts, N=400, PSUM-accum."""
        chunk = chunks[b]
        for k in range(16):
            nc.tensor.matmul(cps[:], fv_slice(k),
                             chunk[:, k * OUT:(k + 1) * OUT],
                             start=(cmm_n[0] == 0), stop=(cmm_n[0] == 63))
            cmm_n[0] += 1

    def xbar(b, st):
        """SBUF->SBUF xbar transpose: st [64 (u,kt'), 128 fp] ->
        feats[:, b*64:(b+1)*64]."""
        nc.sync.dma_start_transpose(feats[:, b * 64:(b + 1) * 64], st[:])

    xv = xdup[:].rearrange("i (u t) -> i u t", u=NUTT)

    # ---- scale h=8: rows (u, ph), cols (pl, q). 32 MMs K=128 M=32 N=32.
    acc = ps.tile([64, 128], FP32, tag="featps")
    x8 = xv.rearrange("i u (ph pl j) -> i u ph pl j", pl=4, j=8)
    for half in range(2):
        for pl in range(4):
            for j0 in range(4):
                nc.tensor.matmul(
                    acc[half * 32:(half + 1) * 32, pl * 32:(pl + 1) * 32],
                    x8[:, 2 * half:2 * half + 2, :, pl, 2 * j0],
                    w8j2[:, j0 * 32:(j0 + 1) * 32],
                    start=(j0 == 0), stop=(j0 == 3))
    st8 = stg.tile([64, 128], FP16, tag="featst")
    nc.vector.tensor_copy(st8[:], acc[:])
    xbar(0, st8)

    # ---- scale h=16: rows (u, ph), cols (pl, q). 16 MMs K=128 M=64 N=64.
    acc = ps.tile([64, 128], FP32, tag="featps")
    x16 = xv.rearrange("i u (ph pl j) -> i u ph pl j", pl=2, j=16)
    for pl in range(2):
        for j0 in range(8):
            nc.tensor.matmul(
                acc[:, pl * 64:(pl + 1) * 64],
                x16[:, :, :, pl, 2 * j0],
                w16j2[:, j0 * 64:(j0 + 1) * 64],
                start=(j0 == 0), stop=(j0 == 7))
    st16 = stg.tile([64, 128], FP16, tag="featst")
    nc.vector.tensor_copy(st16[:], acc[:])
    xbar(1, st16)

    chunks[2] = load_wfc2_chunk(2, sync_dma)

    # ---- scale h=32: rows (u, p), cols q. 16 MMs K=128 M=64 N=128.
    acc = ps.tile([64, 128], FP32, tag="featps")
    x32 = xv.rearrange("i u (p j) -> i u p j", j=32)
    for j0 in range(16):
        nc.tensor.matmul(acc[:], x32[:, :, :, 2 * j0],
                         w32j2[:, j0 * 128:(j0 + 1) * 128],
                         start=(j0 == 0), stop=(j0 == 15))
    st32 = stg.tile([64, 128], FP16, tag="featst")
    nc.vector.tensor_copy(st32[:], acc[:])
    xbar(2, st32)

    # frames stationary: rows 0..63 = x, row 64 = ones (DVE idle window)
    x65 = const.tile([65, W], FP16, tag="x65")
    nc.vector.tensor_copy(x65[0:64, :], xdup[0:64, :])
    nc.vector.memset(x65[64:65, :], 1.0)

    # ---- scale h=64: rows (oh, u, p), cols ol. 64 MMs K=128 M=32 N=128.
    acc = ps.tile([64, 128], FP32, tag="featps")
    x64 = xv.rearrange("i u (p j) -> i u p j", j=64)
    for oh in range(2):
        for j0 in range(32):
            nc.tensor.matmul(
                acc[oh * 32:(oh + 1) * 32, :],
                x64[:, :, :, 2 * j0],
                w64w2[:, j0 * 256 + oh * 128: j0 * 256 + (oh + 1) * 128],
                start=(j0 == 0), stop=(j0 == 31))
    st64 = stg.tile([64, 128], FP16, tag="featst")
    nc.vector.tensor_copy(st64[:], acc[:])
    xbar(3, st64)

    chunks[3] = load_wfc2_chunk(3, scalar_dma)

    # region 3 cols are (oh, u, p): col(u, kt') = (kt'%2)*32 + u*8 + kt'//2
    fv3 = feats[:, 192:256].rearrange("f (oh u p) -> f oh u p", oh=2, u=4)
    # C matmuls in chunk-arrival order: 0 (SWDGE), 2, 3, 1
    cmms(0, lambda k: fv[:, 0, :, k])
    cmms(2, lambda k: fv[:, 2, :, k])
    cmms(3, lambda k: fv3[:, k % 2, :, k // 2])
    cmms(1, lambda k: fv[:, 1, :, k])

    # ---- frames matmul: out = x^T @ Wfc1^T + 1*cconst + onehot_u^T @ C
    csb = stg.tile([NUTT, OUT], FP16, tag="csb")
    nc.vector.tensor_copy(csb[:], cps[:])
    fsb = outp.tile([128, NUTT * 4 * OUT], FP16, tag="framesout")
    for u in range(NUTT):
        for tc_i in range(4):
            fps = psf.tile([128, OUT], FP32, tag="framesps")
            nc.tensor.matmul(
                fps[:],
                x65[:, u * T + tc_i * 128: u * T + (tc_i + 1) * 128],
                rhs65[:, u * OUT:(u + 1) * OUT], start=True, stop=False)
            nc.tensor.matmul(
                fps[:], e4[:, u * 128:(u + 1) * 128], csb[:],
                start=False, stop=True)
            col = (u * 4 + tc_i) * OUT
            if tc_i % 2 == 0:
                nc.vector.tensor_copy(fsb[:, col:col + OUT], fps[:])
            else:
                nc.scalar.activation(fsb[:, col:col + OUT], fps[:],
                                     mybir.ActivationFunctionType.Copy)
    gpsimd_dma(
        bass.AP(tensor=dram["out"], offset=0,
                ap=[[NUTT * 4 * OUT, 128], [1, NUTT * 4 * OUT]]),
        fsb[:])


_NC_CACHE = None


def _get_nc():
    global _NC_CACHE
    if _NC_CACHE is None:
        _NC_CACHE = build_program()
    return _NC_CACHE


# ---------------------------------------------------------------------------
# entry point
# ---------------------------------------------------------------------------

def run(inputs, trace=False, **kw):
    nc = _get_nc()
    prep = host_prep(inputs["W8"], inputs["b8"], inputs["W16"], inputs["b16"],
                     inputs["W32"], inputs["b32"], inputs["W64"], inputs["b64"],
                     inputs["Wfc"], inputs["bfc"])
    batch = np.asarray(inputs["batch"], np.float32)
    in_maps = []
    for c in range(NCORES):
        x4 = batch[NUTT * c:NUTT * (c + 1)].transpose(1, 0, 2).reshape(F, W)
        xw = np.zeros((128, XW_W), dtype=NPF16)
        xw[0:64, 0:W] = x4.astype(NPF16)
        xw[64:128, 0:W - 1] = x4[:, 1:].astype(NPF16)
        xw[:, W:] = prep["wsmall"]
        m = {k: v for k, v in prep.items() if k != "wsmall"}
        m["xw"] = xw
        in_maps.append(m)
    res = run_bass_kernel_spmd(nc, in_maps, core_ids=list(range(NCORES)),
                               trace=trace, **kw)
    # out[t, u, tc, o] -> rows u*512 + tc*128 + t
    out = np.concatenate(
        [r["out"].reshape(128, NUTT, 4, OUT).transpose(1, 2, 0, 3)
         .reshape(NUTT * T, OUT) for r in res.results], axis=0)
    return out.astype(np.float32), res


def kernel(**inputs):
    out, _ = run(inputs)
    return out


# revision 20
# speedup vs baseline: 1.0975x; 1.0008x over previous
"""Trainium2 Bass kernel for nn_CNNModel_82222853915196.

Model (per utterance x: (64, 512)):
  multiscale patch features (h in {8,16,32,64}) -> feats (8192,)
  out[t, :] = Wfc @ concat([x[:, t], feats]) + bfc

Factorization: feats is broadcast over t, so
  out = x.T @ Wfc1.T + 1*(Wfc2 @ feats).T + 1*cconst.T
with all feature-bias terms folded into cconst on the host.

Feature contraction: masked stationary weights over the full 64-row
contraction fuse all row offsets k into the matmul M dim; two within-row
offsets j are fused into K=128 via a second, one-column-shifted copy of x
in SBUF partitions 64..127.

Each scale's feature matmuls are arranged so the staged fp16 tile is
[ (u, kt') rows | fp cols ] -- a pure 2D transpose of the [fp | (u, kt)]
layout the C matmul needs -- so a single SBUF->SBUF xbar-transpose DMA per
scale replaces any DRAM scatter/gather round trip.

C rides into the frames matmul as a second accumulating matmul (K=4,
one-hot utterance selector x C rows), so no cross-partition copies of C
are needed; cconst is a host-prefilled ones-row (K=65 main matmul).

DMA count is kept minimal (the rings serialize at ~2us per dma_start):
the big streams are split over both HWDGE rings, small weights ride in one
merged upload, outputs go out per-utterance on the SWDGE ring.

Everything runs in fp16 except PSUM accumulation (fp32); output is written
fp16 and cast to fp32 on the host. Overall rel err ~5e-4 vs tolerance 2e-2.

Sharding: pure data parallel - 32 utterances -> 8 cores x 4. Weights
replicated; no cross-core communication (collectives have ~10-100us fixed
cost in this environment and are a net loss).
"""

import os
import sys
from contextlib import ExitStack

import numpy as np

for _p in ("/opt/trn_rl_repo", "/root/.axon_site/_ro/trn_rl_repo"):
    if os.path.isdir(_p) and _p not in sys.path:
        sys.path.insert(0, _p)

import concourse.bass as bass
import concourse.tile as tile
from concourse import bacc, mybir
from concourse.bass_utils import run_bass_kernel_spmd

NCORES = 8
NUTT = 4                 # utterances per core
T = 512
F = 64
OUT = 400
W = NUTT * T             # 2048, free width of the x tile
FP32 = mybir.dt.float32
FP16 = mybir.dt.float16
NPF16 = np.float16


# ---------------------------------------------------------------------------
# host-side weight preparation
# ---------------------------------------------------------------------------

def _build_devindex():
    """dev[kt, fp] = reference flat feature index m in [0, 8192).

    Device feats layout (scale regions of 16 kt each):
      h=8 : kt = ph,        fp = pl*32 + k*4 + o    (p = ph*4 + pl)
      h=16: kt = 16 + ph,   fp = pl*64 + k*16 + o   (p = ph*2 + pl)
      h=32: kt = 32 + p,    fp = k*64 + o
      h=64: kt = 48+p*2+oh, fp = ol                 (o = oh*128 + ol)
    """
    dev = np.full((64, 128), -1, dtype=np.int64)
    for ph in range(16):
        for pl in range(4):
            for k in range(8):
                for o in range(4):
                    dev[ph, pl * 32 + k * 4 + o] = (k * 64 + ph * 4 + pl) * 4 + o
    for ph in range(16):
        for pl in range(2):
            for k in range(4):
                for o in range(16):
                    dev[16 + ph, pl * 64 + k * 16 + o] = \
                        2048 + (k * 32 + ph * 2 + pl) * 16 + o
    for p in range(16):
        for k in range(2):
            for o in range(64):
                dev[32 + p, k * 64 + o] = 4096 + (k * 16 + p) * 64 + o
    for p in range(8):
        for o in range(256):
            dev[48 + p * 2 + o // 128, o % 128] = 6144 + p * 256 + o
    assert dev.min() >= 0
    return dev


def _masked2(Wh, nk, h, no):
    """w2[(jo, r), j0*(nk*no) + k*no + o] = Wh[k, o, (r-k)*h + 2*j0 + jo]
    for 0 <= r-k < h, else 0."""
    f32 = np.float32
    w = np.zeros((2, 64, h // 2, nk * no), dtype=f32)
    for k in range(nk):
        Wk = np.asarray(Wh[k], f32).reshape(no, h, h)      # [o, i, j]
        for jo in range(2):
            # [i, j0, o]
            w[jo, k:k + h, :, k * no:(k + 1) * no] = \
                Wk[:, :, jo::2].transpose(1, 2, 0)
    return w.reshape(128, (h // 2) * nk * no)


def host_prep(W8, b8, W16, b16, W32, b32, W64, b64, Wfc, bfc):
    f32 = np.float32
    W64 = np.asarray(W64, f32)
    Wfc = np.asarray(Wfc, f32)
    b8 = np.asarray(b8, f32); b16 = np.asarray(b16, f32)
    b32 = np.asarray(b32, f32); b64 = np.asarray(b64, f32)
    bfc = np.asarray(bfc, f32)

    w8j2 = _masked2(W8, 8, 8, 4)        # [128, 128]
    w16j2 = _masked2(W16, 4, 16, 16)    # [128, 512]
    w32j2 = _masked2(W32, 2, 32, 64)    # [128, 2048]
    wsmall = np.concatenate([w8j2, w16j2, w32j2], axis=1)  # [128, 2688]
    # w64w2[(jo,i), j0*256+o] = W64[o, i*64 + 2*j0 + jo]
    w64w2 = np.ascontiguousarray(
        W64.reshape(256, 64, 32, 2).transpose(3, 1, 2, 0).reshape(128, 8192))

    dev = _build_devindex()
    Wfc2 = Wfc[:, 64:]
    # wfc2c[fp, kt*OUT + o] = Wfc2[o, dev[kt, fp]]
    wfc2c = np.ascontiguousarray(
        Wfc2[:, dev.reshape(-1)].T.reshape(64, 128, OUT)
        .transpose(1, 0, 2).reshape(128, 64 * OUT))

    fb = np.zeros(8192, dtype=np.float64)
    fb[0:2048] = np.broadcast_to(b8[:, None, :], (8, 64, 4)).reshape(-1)
    fb[2048:4096] = np.broadcast_to(b16[:, None, :], (4, 32, 16)).reshape(-1)
    fb[4096:6144] = np.broadcast_to(b32[:, None, :], (2, 16, 64)).reshape(-1)
    fb[6144:8192] = np.broadcast_to(b64[None, :], (8, 256)).reshape(-1)
    cconst = (Wfc2.astype(np.float64) @ fb + bfc.astype(np.float64)).astype(f32)

    # frames moving operand: rows 0..63 = wfc1 (tiled per utt), row 64 = cconst
    wfc1c = np.zeros((65, NUTT * OUT), dtype=f32)
    wfc1c[0:64] = np.tile(Wfc[:, :64].T, (1, NUTT))
    wfc1c[64] = np.tile(cconst, NUTT)

    e4 = np.zeros((NUTT, NUTT * 128), dtype=f32)
    for u in range(NUTT):
        e4[u, u * 128:(u + 1) * 128] = 1.0

    return {
        "e4": e4.astype(NPF16),
        "wsmall": np.ascontiguousarray(wsmall.astype(NPF16)),
        "w64w2": w64w2.astype(NPF16),
        "wfc2c": wfc2c.astype(NPF16),
        "wfc1c": np.ascontiguousarray(wfc1c.astype(NPF16)),
    }


XW_W = W + 2688          # xdup ++ wsmall, merged early upload


# ---------------------------------------------------------------------------
# device program
# ---------------------------------------------------------------------------

def build_program(trace_sim=False):
    nc = bacc.Bacc("TRN2", target_bir_lowering=False, debug=False)

    dram = dict(
        xw=nc.dram_tensor("xw", [128, XW_W], FP16, kind="ExternalInput"),
        w64w2=nc.dram_tensor("w64w2", [128, 8192], FP16, kind="ExternalInput"),
        wfc2c=nc.dram_tensor("wfc2c", [128, 64 * OUT], FP16, kind="ExternalInput"),
        wfc1c=nc.dram_tensor("wfc1c", [65, NUTT * OUT], FP16, kind="ExternalInput"),
        e4=nc.dram_tensor("e4", [NUTT, NUTT * 128], FP16, kind="ExternalInput"),
        out=nc.dram_tensor("out", [128, NUTT * 4 * OUT], FP16, kind="ExternalOutput"),
    )

    with tile.TileContext(nc, trace_sim=trace_sim) as tc:
        with ExitStack() as ctx:
            _emit(nc, tc, ctx, dram)

    nc.compile()
    return nc


def _emit(nc, tc, ctx, dram):
    scalar_dma = nc.scalar.dma_start
    gpsimd_dma = nc.gpsimd.dma_start
    sync_dma = nc.sync.dma_start

    const = ctx.enter_context(tc.tile_pool(name="const", bufs=1))
    stg = ctx.enter_context(tc.tile_pool(name="stg", bufs=2))
    wfc2p = ctx.enter_context(tc.tile_pool(name="wfc2p", bufs=4))
    outp = ctx.enter_context(tc.tile_pool(name="outp", bufs=2))
    ps = ctx.enter_context(tc.tile_pool(name="ps", bufs=2, space="PSUM"))
    psc = ctx.enter_context(tc.tile_pool(name="psc", bufs=1, space="PSUM"))
    psf = ctx.enter_context(tc.tile_pool(name="psf", bufs=3, space="PSUM"))

    CH = 16  # wfc2 kt per streamed chunk (one chunk per scale region)

    # ---- ring layout (HWDGE rings ~130-165 GB/s each; SWDGE ~340 GB/s on
    # big DMAs but ~2us serial setup each):
    #  gpsimd : xw (x + small weights, first), chunk0, chunk1, out (one DMA)
    #  sync   : w64-h1, xbar transposes, chunk2
    #  scalar : w64-h2, wfc1c, e4, chunk3
    xw = const.tile([128, XW_W], FP16, tag="xw")
    sync_dma(xw[0:64, 0:W],
             bass.AP(tensor=dram["xw"], offset=0, ap=[[XW_W, 64], [1, W]]))
    scalar_dma(xw[64:128, 0:W],
               bass.AP(tensor=dram["xw"], offset=64 * XW_W, ap=[[XW_W, 64], [1, W]]))
    gpsimd_dma(xw[:, W:],
               bass.AP(tensor=dram["xw"], offset=W, ap=[[XW_W, 128], [1, 2688]]))
    xdup = xw[:, 0:W]
    w8j2 = xw[:, W:W + 128]
    w16j2 = xw[:, W + 128:W + 640]
    w32j2 = xw[:, W + 640:W + 2688]

    w64w2 = const.tile([128, 8192], FP16, tag="w64w2")
    sync_dma(w64w2[:, 0:4096],
             bass.AP(tensor=dram["w64w2"], offset=0, ap=[[8192, 128], [1, 4096]]))
    scalar_dma(w64w2[:, 4096:8192],
               bass.AP(tensor=dram["w64w2"], offset=4096,
                       ap=[[8192, 128], [1, 4096]]))

    # frames moving operand: rows 0..63 wfc1, row 64 cconst (host-prefilled)
    rhs65 = const.tile([65, NUTT * OUT], FP16, tag="rhs65")
    scalar_dma(rhs65[:], dram["wfc1c"].ap())

    def load_wfc2_half(hf, dma):
        half = wfc2p.tile([128, 32 * OUT], FP16, tag="wfc2half", bufs=2)
        dma(half[:],
            bass.AP(tensor=dram["wfc2c"], offset=hf * 32 * OUT,
                    ap=[[64 * OUT, 128], [1, 32 * OUT]]))
        return half

    halves = [None, None]
    halves[0] = load_wfc2_half(0, sync_dma)
    halves[1] = load_wfc2_half(1, scalar_dma)
    chunks = [halves[0][:, 0:CH * OUT], halves[0][:, CH * OUT:2 * CH * OUT],
              halves[1][:, 0:CH * OUT], halves[1][:, CH * OUT:2 * CH * OUT]]

    # one-hot utterance selector for the C-broadcast matmul (K=4)
    e4 = const.tile([NUTT, NUTT * 128], FP16, tag="e4")
    scalar_dma(e4[:], dram["e4"].ap())

    feats = const.tile([128, 256], FP16, tag="feats")
    fv = feats.rearrange("f (s u k) -> f s u k", s=4, u=4)
    cps = psc.tile([NUTT, OUT], FP32, tag="cps")

    cmm_n = [0]

    def cmms(b, fv_slice):
        """C matmuls for region b: 16 kts, M=4 utts, N=400, PSUM-accum."""
        chunk = chunks[b]
        for k in range(16):
            nc.tensor.matmul(cps[:], fv_slice(k),
                             chunk[:, k * OUT:(k + 1) * OUT],
                             start=(cmm_n[0] == 0), stop=(cmm_n[0] == 63))
            cmm_n[0] += 1

    def xbar(b, st):
        """SBUF->SBUF xbar transpose: st [64 (u,kt'), 128 fp] ->
        feats[:, b*64:(b+1)*64]."""
        nc.sync.dma_start_transpose(feats[:, b * 64:(b + 1) * 64], st[:])

    xv = xdup[:].rearrange("i (u t) -> i u t", u=NUTT)

    # ---- scale h=8: rows (u, ph), cols (pl, q). 32 MMs K=128 M=32 N=32.
    acc = ps.tile([64, 128], FP32, tag="featps")
    x8 = xv.rearrange("i u (ph pl j) -> i u ph pl j", pl=4, j=8)
    for half in range(2):
        for pl in range(4):
            for j0 in range(4):
                nc.tensor.matmul(
                    acc[half * 32:(half + 1) * 32, pl * 32:(pl + 1) * 32],
                    x8[:, 2 * half:2 * half + 2, :, pl, 2 * j0],
                    w8j2[:, j0 * 32:(j0 + 1) * 32],
                    start=(j0 == 0), stop=(j0 == 3))
    st8 = stg.tile([64, 128], FP16, tag="featst")
    nc.vector.tensor_copy(st8[:], acc[:])
    xbar(0, st8)

    # ---- scale h=16: rows (u, ph), cols (pl, q). 16 MMs K=128 M=64 N=64.
    acc = ps.tile([64, 128], FP32, tag="featps")
    x16 = xv.rearrange("i u (ph pl j) -> i u ph pl j", pl=2, j=16)
    for pl in range(2):
        for j0 in range(8):
            nc.tensor.matmul(
                acc[:, pl * 64:(pl + 1) * 64],
                x16[:, :, :, pl, 2 * j0],
                w16j2[:, j0 * 64:(j0 + 1) * 64],
                start=(j0 == 0), stop=(j0 == 7))
    st16 = stg.tile([64, 128], FP16, tag="featst")
    nc.vector.tensor_copy(st16[:], acc[:])
    xbar(1, st16)


    # ---- scale h=32: rows (u, p), cols q. 16 MMs K=128 M=64 N=128.
    acc = ps.tile([64, 128], FP32, tag="featps")
    x32 = xv.rearrange("i u (p j) -> i u p j", j=32)
    for j0 in range(16):
        nc.tensor.matmul(acc[:], x32[:, :, :, 2 * j0],
                         w32j2[:, j0 * 128:(j0 + 1) * 128],
                         start=(j0 == 0), stop=(j0 == 15))
    st32 = stg.tile([64, 128], FP16, tag="featst")
    nc.vector.tensor_copy(st32[:], acc[:])
    xbar(2, st32)

    # frames stationary: rows 0..63 = x, row 64 = ones (DVE idle window)
    x65 = const.tile([65, W], FP16, tag="x65")
    nc.vector.tensor_copy(x65[0:64, :], xdup[0:64, :])
    nc.vector.memset(x65[64:65, :], 1.0)

    # ---- scale h=64: rows (oh, u, p), cols ol. 64 MMs K=128 M=32 N=128.
    acc = ps.tile([64, 128], FP32, tag="featps")
    x64 = xv.rearrange("i u (p j) -> i u p j", j=64)
    for oh in range(2):
        for j0 in range(32):
            nc.tensor.matmul(
                acc[oh * 32:(oh + 1) * 32, :],
                x64[:, :, :, 2 * j0],
                w64w2[:, j0 * 256 + oh * 128: j0 * 256 + (oh + 1) * 128],
                start=(j0 == 0), stop=(j0 == 31))
    st64 = stg.tile([64, 128], FP16, tag="featst")
    nc.vector.tensor_copy(st64[:], acc[:])
    xbar(3, st64)


    # region 3 cols are (oh, u, p): col(u, kt') = (kt'%2)*32 + u*8 + kt'//2
    fv3 = feats[:, 192:256].rearrange("f (oh u p) -> f oh u p", oh=2, u=4)
    cmms(0, lambda k: fv[:, 0, :, k])
    cmms(1, lambda k: fv[:, 1, :, k])
    cmms(2, lambda k: fv[:, 2, :, k])
    cmms(3, lambda k: fv3[:, k % 2, :, k // 2])

    # ---- frames matmul: out = x^T @ Wfc1^T + 1*cconst + onehot_u^T @ C
    csb = stg.tile([NUTT, OUT], FP16, tag="csb")
    nc.vector.tensor_copy(csb[:], cps[:])
    fsb = outp.tile([128, NUTT * 4 * OUT], FP16, tag="framesout")
    for u in range(NUTT):
        for tc_i in range(4):
            fps = psf.tile([128, OUT], FP32, tag="framesps")
            nc.tensor.matmul(
                fps[:],
                x65[:, u * T + tc_i * 128: u * T + (tc_i + 1) * 128],
                rhs65[:, u * OUT:(u + 1) * OUT], start=True, stop=False)
            nc.tensor.matmul(
                fps[:], e4[:, u * 128:(u + 1) * 128], csb[:],
                start=False, stop=True)
            col = (u * 4 + tc_i) * OUT
            if tc_i % 2 == 0:
                nc.vector.tensor_copy(fsb[:, col:col + OUT], fps[:])
            else:
                nc.scalar.activation(fsb[:, col:col + OUT], fps[:],
                                     mybir.ActivationFunctionType.Copy)
    sync_dma(
        bass.AP(tensor=dram["out"], offset=0,
                ap=[[NUTT * 4 * OUT, 128], [1, 2 * 4 * OUT]]),
        fsb[:, 0:2 * 4 * OUT])
    scalar_dma(
        bass.AP(tensor=dram["out"], offset=2 * 4 * OUT,
                ap=[[NUTT * 4 * OUT, 128], [1, 2 * 4 * OUT]]),
        fsb[:, 2 * 4 * OUT:])


_NC_CACHE = None


def _get_nc():
    global _NC_CACHE
    if _NC_CACHE is None:
        _NC_CACHE = build_program()
    return _NC_CACHE


# ---------------------------------------------------------------------------
# entry point
# ---------------------------------------------------------------------------

def run(inputs, trace=False, **kw):
    nc = _get_nc()
    prep = host_prep(inputs["W8"], inputs["b8"], inputs["W16"], inputs["b16"],
                     inputs["W32"], inputs["b32"], inputs["W64"], inputs["b64"],
                     inputs["Wfc"], inputs["bfc"])
    batch = np.asarray(inputs["batch"], np.float32)
    in_maps = []
    for c in range(NCORES):
        x4 = batch[NUTT * c:NUTT * (c + 1)].transpose(1, 0, 2).reshape(F, W)
        xw = np.zeros((128, XW_W), dtype=NPF16)
        xw[0:64, 0:W] = x4.astype(NPF16)
        xw[64:128, 0:W - 1] = x4[:, 1:].astype(NPF16)
        xw[:, W:] = prep["wsmall"]
        m = {k: v for k, v in prep.items() if k != "wsmall"}
        m["xw"] = xw
        in_maps.append(m)
    res = run_bass_kernel_spmd(nc, in_maps, core_ids=list(range(NCORES)),
                               trace=trace, **kw)
    # out[t, u, tc, o] -> rows u*512 + tc*128 + t
    out = np.concatenate(
        [r["out"].reshape(128, NUTT, 4, OUT).transpose(1, 2, 0, 3)
         .reshape(NUTT * T, OUT) for r in res.results], axis=0)
    return out.astype(np.float32), res


def kernel(**inputs):
    out, _ = run(inputs)
    return out
